# revision 6
# baseline (speedup 1.0000x reference)
"""Mixtral sparse-MoE block (dense formulation) on 8 trn2 NeuronCores.

Expert-parallel: core e computes expert e over ALL tokens:
    h1 = silu(X @ w1[e]);  h2 = X @ w2[e];  g = h1*h2        [T, I]
    amax_e = max(g)
    out_e  = (g @ w3[e]) * cw[:, e, None]                    [T, H]
Host: final = sum_e out_e ; amax = stack(amax_e).

Device layout (per core): all matmuls keep the contraction dim on SBUF
partitions, so X is fed pre-transposed (host provides xT = X.T):
    mm1/mm2: psum[i(128), t(512)] += w1[hs](128h,128i).T @ xT[hs](128h,512t)
    gate:    g^T[i(128), IB, T_BLK] = silu(h1)*h2  (+ running amax)
    mm3:     psum[t(128), h(512)]  += g^T[ib,ts](128i,128t).T @ w3[ib](128i,512h)
"""

import os

import numpy as np
import ml_dtypes

import bass_rust
import concourse.bacc as bacc
import concourse.bass as bass
import concourse.mybir as mybir
from concourse.tile import TileContext
from concourse.bass_utils import run_bass_kernel_spmd

P = 128
E = 8

# mode: "f32r" (fp32 data, fast-fp32 matmuls) or "bf16" (bf16 data/matmuls)
MODE = os.environ.get("MOE_MODE", "f32r")


def _dt(mode):
    # float32r = fast-fp32 matmul dtype (same 4 bytes; walrus requires the
    # dtype to be declared fp32r end-to-end for operands of fp32r matmuls)
    return mybir.dt.float32r if mode == "f32r" else mybir.dt.bfloat16


def _np_dt(mode):
    return np.float32 if mode == "f32r" else ml_dtypes.bfloat16


def _mm_cast(ap, mode):
    return ap


def build_nc(T, H, I, mode=MODE, T_BLK=None):
    """Build the single-core Bass program (same NEFF for all cores)."""
    io_dt = _dt(mode)
    f32 = mybir.dt.float32

    if T_BLK is None:
        T_BLK = 512 if mode == "f32r" else 1024
    T_BLK = min(T_BLK, T)

    HS = H // P              # h sub-tiles (contraction mm1/2)
    IB = I // P              # i blocks (g partitions / contraction mm3)
    NT = T // T_BLK          # token blocks
    TS = T_BLK // P          # 128-token sub-blocks per t-block
    N12 = min(512, T_BLK)    # mm1/2 moving free size
    TF = T_BLK // N12        # psum tiles per (ib, t-block) per matmul
    N3 = min(512, H)         # mm3 moving free size
    NHB = H // N3            # mm3 output column blocks
    W3S = min(8 if mode == "f32r" else 16, IB)   # w3 slab depth (i-blocks per DMA)

    assert T % T_BLK == 0 and T_BLK % N12 == 0 and H % N3 == 0
    assert H % P == 0 and I % P == 0 and IB % W3S == 0

    nc = bacc.Bacc("TRN2", target_bir_lowering=False)

    xT = nc.dram_tensor("xT", [H, T], io_dt, kind="ExternalInput")
    w1 = nc.dram_tensor("w1", [H, I], io_dt, kind="ExternalInput")
    w2 = nc.dram_tensor("w2", [H, I], io_dt, kind="ExternalInput")
    w3 = nc.dram_tensor("w3", [I, H], io_dt, kind="ExternalInput")
    cw = nc.dram_tensor("cw", [P, T // P], f32, kind="ExternalInput")
    out = nc.dram_tensor("out", [T, H], f32, kind="ExternalOutput")
    amax = nc.dram_tensor("amax", [P, 1], f32, kind="ExternalOutput")

    xT_r = xT.rearrange("(hs hp) t -> hp hs t", hp=P)
    w1_r = w1.rearrange("(hs hp) i -> hp hs i", hp=P)
    w2_r = w2.rearrange("(hs hp) i -> hp hs i", hp=P)
    w3_r = w3.rearrange("(ib ip) h -> ip ib h", ip=P)
    out_r = out.rearrange("(tk tp) h -> tp tk h", tp=P)

    with TileContext(nc) as tc:
        with (
            tc.tile_pool(name="const", bufs=1) as const,
            tc.tile_pool(name="xt", bufs=1) as xt_pool,
            tc.tile_pool(name="g", bufs=1) as g_pool,
            tc.tile_pool(name="w12", bufs=2) as w12_pool,
            tc.tile_pool(name="w3", bufs=2) as w3_pool,
            tc.tile_pool(name="h1", bufs=4) as h_pool,
            tc.tile_pool(name="o", bufs=4) as o_pool,
            tc.tile_pool(name="r", bufs=4) as r_pool,
            tc.tile_pool(name="ps12", bufs=4, space="PSUM") as ps12,
            tc.tile_pool(name="ps3", bufs=4, space="PSUM") as ps3,
        ):
            cw_sb = const.tile([P, T // P], f32)
            nc.sync.dma_start(cw_sb[:], cw[:, :])
            amax_acc = const.tile([P, 1], f32)
            nc.vector.memset(amax_acc[:], -3.0e38)

            for tb in range(NT):
                t0 = tb * T_BLK
                xt_t = xt_pool.tile([P, HS, T_BLK], io_dt, tag="xt")
                nc.sync.dma_start(xt_t[:], xT_r[:, :, t0 : t0 + T_BLK])
                g_t = g_pool.tile([P, IB, T_BLK], io_dt, tag="g")

                # ---- mm1/mm2 + gating: g^T[i, t] for this token block ----
                for ib in range(IB):
                    w1_t = w12_pool.tile([P, HS, P], io_dt, tag="w1")
                    nc.sync.dma_start(w1_t[:], w1_r[:, :, ib * P : (ib + 1) * P])
                    w2_t = w12_pool.tile([P, HS, P], io_dt, tag="w2")
                    nc.sync.dma_start(w2_t[:], w2_r[:, :, ib * P : (ib + 1) * P])
                    for tf in range(TF):
                        ts0 = tf * N12
                        ps1 = ps12.tile([P, N12], f32, tag="ps12")
                        for hs in range(HS):
                            nc.tensor.matmul(
                                ps1[:],
                                lhsT=_mm_cast(w1_t[:, hs], mode),
                                rhs=_mm_cast(xt_t[:, hs, ts0 : ts0 + N12], mode),
                                start=(hs == 0),
                                stop=(hs == HS - 1),
                            )
                        ps2 = ps12.tile([P, N12], f32, tag="ps12")
                        for hs in range(HS):
                            nc.tensor.matmul(
                                ps2[:],
                                lhsT=_mm_cast(w2_t[:, hs], mode),
                                rhs=_mm_cast(xt_t[:, hs, ts0 : ts0 + N12], mode),
                                start=(hs == 0),
                                stop=(hs == HS - 1),
                            )
                        sg = h_pool.tile([P, N12], f32, tag="sg")
                        nc.scalar.activation(
                            sg[:], ps1[:], mybir.ActivationFunctionType.Sigmoid
                        )
                        h1 = h_pool.tile([P, N12], f32, tag="h1")
                        nc.vector.tensor_mul(h1[:], sg[:], ps1[:])
                        rmax = r_pool.tile([P, 1], f32, tag="rmax")
                        nc.vector.tensor_mul(
                            g_t[:, ib, ts0 : ts0 + N12], h1[:], ps2[:]
                        )
                        nc.vector.reduce_max(
                            rmax[:],
                            g_t[:, ib, ts0 : ts0 + N12],
                            axis=bass_rust.AxisListType.X,
                        )
                        nc.vector.tensor_max(amax_acc[:], amax_acc[:], rmax[:])

                # ---- mm3: out[t, h] for this token block ----
                for hb in range(NHB):
                    h0 = hb * N3
                    pso = [
                        ps3.tile([P, N3], f32, tag="ps3", name=f"pso_{ts}")
                        for ts in range(TS)
                    ]
                    for sl in range(IB // W3S):
                        w3_t = w3_pool.tile([P, W3S, N3], io_dt, tag="w3")
                        nc.sync.dma_start(
                            w3_t[:],
                            w3_r[:, sl * W3S : (sl + 1) * W3S, h0 : h0 + N3],
                        )
                        for ts in range(TS):
                            for k in range(W3S):
                                ib = sl * W3S + k
                                nc.tensor.matmul(
                                    pso[ts][:],
                                    lhsT=_mm_cast(
                                        g_t[:, ib, ts * P : (ts + 1) * P], mode
                                    ),
                                    rhs=_mm_cast(w3_t[:, k], mode),
                                    start=(ib == 0),
                                    stop=(ib == IB - 1),
                                )
                    for ts in range(TS):
                        o_t = o_pool.tile([P, N3], f32, tag="o")
                        col = tb * TS + ts
                        nc.vector.tensor_scalar_mul(
                            o_t[:], pso[ts][:], cw_sb[:, col : col + 1]
                        )
                        nc.sync.dma_start(out_r[:, col, h0 : h0 + N3], o_t[:])

            nc.sync.dma_start(amax[:, :], amax_acc[:])

    nc.finalize()
    return nc


_nc_cache = {}


def _get_nc(T, H, I, mode):
    key = (T, H, I, mode)
    if key not in _nc_cache:
        _nc_cache[key] = build_nc(T, H, I, mode)
    return _nc_cache[key]


last_result = None  # BassKernelResults of the most recent run (for profiling)


def kernel(hidden_states, routing_weights, w1, w2, w3, selected_experts):
    global last_result
    mode = MODE
    hidden_states = np.asarray(hidden_states, dtype=np.float32)
    routing_weights = np.asarray(routing_weights, dtype=np.float32)
    w1 = np.asarray(w1, dtype=np.float32)
    w2 = np.asarray(w2, dtype=np.float32)
    w3 = np.asarray(w3, dtype=np.float32)
    sel = np.asarray(selected_experts)

    T, H = hidden_states.shape
    num_experts, _, I = w1.shape
    assert num_experts == E

    np_dt = _np_dt(mode)
    xT = np.ascontiguousarray(hidden_states.T).astype(np_dt, copy=False)

    # combine weights cw[t, e] = sum_k routing_weights[t, k] * [sel[t, k] == e]
    cw = np.zeros((T, E), np.float32)
    for k in range(sel.shape[1]):
        np.add.at(cw, (np.arange(T), sel[:, k].astype(np.int64)), routing_weights[:, k])
    # tile for per-partition broadcast: cw_t[e][tp, tk] = cw[tk*128 + tp, e]
    cw_t = np.ascontiguousarray(cw.reshape(T // P, P, E).transpose(2, 1, 0))

    in_maps = []
    for e in range(E):
        in_maps.append(
            {
                "xT": xT,
                "w1": np.ascontiguousarray(w1[e]).astype(np_dt, copy=False),
                "w2": np.ascontiguousarray(w2[e]).astype(np_dt, copy=False),
                "w3": np.ascontiguousarray(w3[e]).astype(np_dt, copy=False),
                "cw": np.ascontiguousarray(cw_t[e]),
            }
        )

    nc = _get_nc(T, H, I, mode)
    trace = bool(int(os.environ.get("MOE_TRACE", "0")))
    res = run_bass_kernel_spmd(
        nc, in_maps, core_ids=list(range(E)), trace=trace
    )
    last_result = res

    final = np.zeros((T, H), np.float32)
    for r in res.results:
        final += r["out"]
    amax = np.stack([r["amax"].max() for r in res.results]).astype(np.float32)
    return final, amax


# revision 17
# speedup vs baseline: 1.5184x; 1.5184x over previous
"""Mixtral sparse-MoE block (dense formulation) on 8 trn2 NeuronCores.

Expert-parallel: core e computes expert e over ALL tokens:
    h1 = silu(X @ w1[e]);  h2 = X @ w2[e];  g = h1*h2        [T, I]
    amax_e = max(g)
    out_e  = (g @ w3[e]) * cw[:, e, None]                    [T, H]
Host: final = sum_e out_e ; amax = stack(amax_e).

Device layout (per core): all matmuls keep the contraction dim on SBUF
partitions, so X is fed pre-transposed (host provides xT = X.T):
    mm1/mm2: psum[i(128), t(512)] += w1[hs](128h,128i).T @ xT[hs](128h,512t)
    gate:    g^T[i(128), IB, T_BLK] = silu(h1)*h2  (+ running amax)
    mm3:     psum[t(128), h(512)]  += g^T[ib,ts](128i,128t).T @ w3[ib](128i,512h)
"""

import os

import numpy as np
import ml_dtypes

import bass_rust
import concourse.bacc as bacc
import concourse.bass as bass
import concourse.mybir as mybir
from concourse.tile import TileContext
from concourse.bass_utils import run_bass_kernel_spmd

P = 128
E = 8

# mode: "f32r" (fp32 data, fast-fp32 matmuls) or "bf16" (bf16 data/matmuls)
MODE = os.environ.get("MOE_MODE", "f32r")


def _dt(mode):
    # float32r = fast-fp32 matmul dtype (same 4 bytes; walrus requires the
    # dtype to be declared fp32r end-to-end for operands of fp32r matmuls)
    if mode == "f32r":
        return mybir.dt.float32r
    if mode == "f16":
        return mybir.dt.float16
    return mybir.dt.bfloat16


def _np_dt(mode):
    if mode == "f32r":
        return np.float32
    if mode == "f16":
        return np.float16
    return ml_dtypes.bfloat16


def _mm_cast(ap, mode):
    return ap


def build_nc(T, H, I, mode=MODE, T_BLK=None, T_SEL=None):
    """Build the single-core Bass program (same NEFF for all cores).

    T_SEL: only the first T_SEL (permuted) tokens go through mm3/output —
    the host puts each expert's selected tokens first; the rest have
    combine-weight 0 and contribute exactly nothing to the final output.
    mm1/mm2 (and amax) still cover all T tokens.
    """
    io_dt = _dt(mode)
    f32 = mybir.dt.float32

    if T_BLK is None:
        T_BLK = 512 if mode in ("f32r", "f16") else 1024
    T_BLK = min(T_BLK, T)
    if T_SEL is None:
        T_SEL = T
    assert T_SEL % T_BLK == 0

    HS = H // P              # h sub-tiles (contraction mm1/2)
    IB = I // P              # i blocks (g partitions / contraction mm3)
    NT = T // T_BLK          # token blocks
    TS = T_BLK // P          # 128-token sub-blocks per t-block
    N12 = min(512, T_BLK)    # mm1/2 moving free size
    TF = T_BLK // N12        # psum tiles per (ib, t-block) per matmul
    N3 = min(512, H)         # mm3 moving free size
    NHB = H // N3            # mm3 output column blocks
    W3S = min(8 if mode == "f32r" else 16, IB)  # w3 slab depth (i-blocks per DMA)

    assert T % T_BLK == 0 and T_BLK % N12 == 0 and H % N3 == 0
    assert H % P == 0 and I % P == 0 and IB % W3S == 0

    nc = bacc.Bacc("TRN2", target_bir_lowering=False)

    xT = nc.dram_tensor("xT", [H, T], io_dt, kind="ExternalInput")
    w1 = nc.dram_tensor("w1", [H, I], io_dt, kind="ExternalInput")
    w2 = nc.dram_tensor("w2", [H, I], io_dt, kind="ExternalInput")
    w3 = nc.dram_tensor("w3", [I, H], io_dt, kind="ExternalInput")
    cw = nc.dram_tensor("cw", [P, T_SEL // P], f32, kind="ExternalInput")
    out = nc.dram_tensor("out", [T_SEL, H], f32, kind="ExternalOutput")
    amax = nc.dram_tensor("amax", [P, 1], f32, kind="ExternalOutput")

    xT_r = xT.rearrange("(hs hp) t -> hp hs t", hp=P)
    w1_r = w1.rearrange("(hs hp) i -> hp hs i", hp=P)
    w2_r = w2.rearrange("(hs hp) i -> hp hs i", hp=P)
    w3_r = w3.rearrange("(ib ip) h -> ip ib h", ip=P)
    out_r = out.rearrange("(tk tp) h -> tp tk h", tp=P)

    with TileContext(nc) as tc:
        with (
            tc.tile_pool(name="const", bufs=1) as const,
            tc.tile_pool(name="xt", bufs=1) as xt_pool,
            tc.tile_pool(name="g", bufs=1) as g_pool,
            tc.tile_pool(name="w12", bufs=2) as w12_pool,
            tc.tile_pool(name="w3", bufs=2) as w3_pool,
            tc.tile_pool(name="h1", bufs=4) as h_pool,
            tc.tile_pool(name="o", bufs=4) as o_pool,
            tc.tile_pool(name="r", bufs=4) as r_pool,
            tc.tile_pool(name="ps12", bufs=4, space="PSUM") as ps12,
            tc.tile_pool(name="ps3", bufs=4, space="PSUM") as ps3,
        ):
            cw_sb = const.tile([P, T_SEL // P], f32)
            nc.sync.dma_start(cw_sb[:], cw[:, :])
            amax_acc = const.tile([P, 1], f32)
            nc.vector.memset(amax_acc[:], -3.0e38)

            for tb in range(NT):
                t0 = tb * T_BLK
                xt_t = xt_pool.tile([P, HS, T_BLK], io_dt, tag="xt")
                nc.sync.dma_start(xt_t[:], xT_r[:, :, t0 : t0 + T_BLK])
                g_t = g_pool.tile([P, IB, T_BLK], io_dt, tag="g")

                # ---- mm1/mm2 + gating: g^T[i, t] for this token block ----
                for ib in range(IB):
                    w1_t = w12_pool.tile([P, HS, P], io_dt, tag="w1")
                    nc.sync.dma_start(w1_t[:], w1_r[:, :, ib * P : (ib + 1) * P])
                    w2_t = w12_pool.tile([P, HS, P], io_dt, tag="w2")
                    nc.sync.dma_start(w2_t[:], w2_r[:, :, ib * P : (ib + 1) * P])
                    for tf in range(TF):
                        ts0 = tf * N12
                        ps1 = ps12.tile([P, N12], f32, tag="ps12")
                        for hs in range(HS):
                            nc.tensor.matmul(
                                ps1[:],
                                lhsT=_mm_cast(w1_t[:, hs], mode),
                                rhs=_mm_cast(xt_t[:, hs, ts0 : ts0 + N12], mode),
                                start=(hs == 0),
                                stop=(hs == HS - 1),
                            )
                        ps2 = ps12.tile([P, N12], f32, tag="ps12")
                        for hs in range(HS):
                            nc.tensor.matmul(
                                ps2[:],
                                lhsT=_mm_cast(w2_t[:, hs], mode),
                                rhs=_mm_cast(xt_t[:, hs, ts0 : ts0 + N12], mode),
                                start=(hs == 0),
                                stop=(hs == HS - 1),
                            )
                        sg = h_pool.tile([P, N12], f32, tag="sg")
                        nc.scalar.activation(
                            sg[:], ps1[:], mybir.ActivationFunctionType.Sigmoid
                        )
                        h1 = h_pool.tile([P, N12], f32, tag="h1")
                        nc.vector.tensor_mul(h1[:], sg[:], ps1[:])
                        rmax = r_pool.tile([P, 1], f32, tag="rmax")
                        nc.vector.tensor_mul(
                            g_t[:, ib, ts0 : ts0 + N12], h1[:], ps2[:]
                        )
                        nc.vector.reduce_max(
                            rmax[:],
                            g_t[:, ib, ts0 : ts0 + N12],
                            axis=bass_rust.AxisListType.X,
                        )
                        nc.vector.tensor_max(amax_acc[:], amax_acc[:], rmax[:])

                # ---- mm3: out[t, h] for this token block ----
                if t0 >= T_SEL:
                    continue  # combine weight is 0 for these tokens
                for hb in range(NHB):
                    h0 = hb * N3
                    pso = [
                        ps3.tile([P, N3], f32, tag="ps3", name=f"pso_{ts}")
                        for ts in range(TS)
                    ]
                    for sl in range(IB // W3S):
                        w3_t = w3_pool.tile([P, W3S, N3], io_dt, tag="w3")
                        nc.sync.dma_start(
                            w3_t[:],
                            w3_r[:, sl * W3S : (sl + 1) * W3S, h0 : h0 + N3],
                        )
                        for ts in range(TS):
                            for k in range(W3S):
                                ib = sl * W3S + k
                                nc.tensor.matmul(
                                    pso[ts][:],
                                    lhsT=_mm_cast(
                                        g_t[:, ib, ts * P : (ts + 1) * P], mode
                                    ),
                                    rhs=_mm_cast(w3_t[:, k], mode),
                                    start=(ib == 0),
                                    stop=(ib == IB - 1),
                                )
                    for ts in range(TS):
                        o_t = o_pool.tile([P, N3], f32, tag="o")
                        col = tb * TS + ts
                        nc.vector.tensor_scalar_mul(
                            o_t[:], pso[ts][:], cw_sb[:, col : col + 1]
                        )
                        nc.sync.dma_start(out_r[:, col, h0 : h0 + N3], o_t[:])

            nc.sync.dma_start(amax[:, :], amax_acc[:])

    nc.finalize()
    return nc


_nc_cache = {}


def _round_up(x, m):
    return ((x + m - 1) // m) * m


def _get_nc(T, H, I, mode, T_SEL):
    key = (T, H, I, mode, T_SEL)
    if key not in _nc_cache:
        _nc_cache[key] = build_nc(T, H, I, mode, T_SEL=T_SEL)
    return _nc_cache[key]


last_result = None  # BassKernelResults of the most recent run (for profiling)


def kernel(hidden_states, routing_weights, w1, w2, w3, selected_experts):
    global last_result
    mode = MODE
    hidden_states = np.asarray(hidden_states, dtype=np.float32)
    routing_weights = np.asarray(routing_weights, dtype=np.float32)
    w1 = np.asarray(w1, dtype=np.float32)
    w2 = np.asarray(w2, dtype=np.float32)
    w3 = np.asarray(w3, dtype=np.float32)
    sel = np.asarray(selected_experts)

    T, H = hidden_states.shape
    num_experts, _, I = w1.shape
    assert num_experts == E

    np_dt = _np_dt(mode)
    xT = np.ascontiguousarray(hidden_states.T).astype(np_dt, copy=False)

    # combine weights cw[t, e] = sum_k routing_weights[t, k] * [sel[t, k] == e]
    cw = np.zeros((T, E), np.float32)
    for k in range(sel.shape[1]):
        np.add.at(cw, (np.arange(T), sel[:, k].astype(np.int64)), routing_weights[:, k])

    # Per-expert token permutation: tokens with cw != 0 first. Only the
    # first T_SEL permuted tokens are run through mm3 on device.
    blk = 512 if mode in ("f32r", "f16") else 1024
    T_SEL = min(T, _round_up(max(1, int(np.max((cw != 0).sum(0)))), blk))
    perms = []
    for e in range(E):
        s = np.nonzero(cw[:, e])[0]
        rest = np.nonzero(cw[:, e] == 0)[0]
        perms.append(np.concatenate([s, rest]))

    in_maps = []
    for e in range(E):
        p = perms[e]
        cw_p = cw[p[:T_SEL], e]
        cw_t = np.ascontiguousarray(cw_p.reshape(T_SEL // P, P).T)
        in_maps.append(
            {
                "xT": np.ascontiguousarray(xT[:, p]),
                "w1": np.ascontiguousarray(w1[e]).astype(np_dt, copy=False),
                "w2": np.ascontiguousarray(w2[e]).astype(np_dt, copy=False),
                "w3": np.ascontiguousarray(w3[e]).astype(np_dt, copy=False),
                "cw": cw_t,
            }
        )

    nc = _get_nc(T, H, I, mode, T_SEL)
    trace = bool(int(os.environ.get("MOE_TRACE", "0")))
    res = run_bass_kernel_spmd(
        nc, in_maps, core_ids=list(range(E)), trace=trace
    )
    last_result = res

    final = np.zeros((T, H), np.float32)
    for e, r in enumerate(res.results):
        final[perms[e][:T_SEL]] += r["out"]
    amax = np.stack([r["amax"].max() for r in res.results]).astype(np.float32)
    return final, amax
